# revision 18
# baseline (speedup 1.0000x reference)
"""Trainium2 Bass kernel for nn_BipropLinear (topk column-masked sign-binarized linear).

Full inputs -> full outputs. Internally sharded over 8 NeuronCores on a 2x4 grid:
  - sample rows (batch*seq = 8192) split 2 ways  (im = 0, 1)
  - output columns (d_out = 4096) split 4 ways   (jn = 0..3)

score/weight are passed to every core in full, but ROW-ROTATED so that each
core's own d_out shard (the jn*1024 rows it binarizes for its out-block and
associate_memory shard) sits at rows [0, 1024). Column statistics are
row-order invariant, so every core computes identical col_mean / scaling
locally -- no collective needed.

Device-side computation per core:
  1. col-sum of score and abs-col-sum of weight (DVE/ACT accumulate over 32
     row-tiles + one fp32/bf16 ones-matmul partition reduce).
  2. threshold for the n_drop=819 smallest column means via a 10-round
     16-ary bisection on count(x <= t) (DVE compares + gpsimd
     partition_all_reduce), with a 1-ulp-stall guard select.
  3. mask, scaling factor sum(|w|*mask)/sum(mask), sign(weight) (shard rows),
     associate_memory = sign(w) * ((mask - score) + score)   [bitwise-matches
     the reference's straight-through perturbation].
  4. out-block = scaling * sample_shard @ (sign(w)*mask)^T as a
     split-precision matmul: sample = hi(fp16) + lo(bf16) via PE transpose +
     split, sign matrix exact in bf16 (PE-transposed on chip), fp32 PSUM
     accumulation -> ~1e-6 relative error vs the fp32 reference.
"""

import numpy as np

import concourse.bass as bass
import concourse.bass_isa as bass_isa
import concourse.mybir as mybir
import concourse.tile as tile
from concourse import bacc
from concourse.bass_utils import run_bass_kernel_spmd
from concourse.masks import make_identity

F32 = mybir.dt.float32
BF16 = mybir.dt.bfloat16
F16 = mybir.dt.float16
ALU = mybir.AluOpType
ACTF = mybir.ActivationFunctionType

B, S, D_IN, D_OUT = 4, 2048, 4096, 4096
GM, GN = 2, 4
M = B * S             # 8192
M_SH = M // GM        # 4096 sample rows per core
O_SH = D_OUT // GN    # 1024 d_out rows per core's shard
N_DROP = 819          # d_in - ceil((1-0.2)*d_in)
N_KEEP = D_IN - N_DROP
KT = D_IN // 128      # 32 k-tiles
MT = M_SH // 128      # 32 m-tiles
OTS = O_SH // 128     # 8 o-subtiles in the shard
WTS = D_OUT // 128    # 32 weight/score row-tiles (full)

# bisection constants: |score| < 1e-3 so |col_mean| < 1e-3.
BIS_LO = -1.1e-3
BIS_RANGE = 2.2e-3
BIS_ROUNDS = 10
BIS_WAYS = 16
GUARD_BUMP = BIS_RANGE * 2.0 ** -25

_NC = None


def _build(ablate=()):
    ablate = set(ablate)
    reps = 1
    for a in ablate:
        if a.startswith("rep"):
            reps = int(a[3:])

    nc = bacc.Bacc("TRN2", target_bir_lowering=False, debug=False,
                   num_devices=8)

    sample = nc.dram_tensor("sample", [M_SH, D_IN], F32, kind="ExternalInput")
    weight = nc.dram_tensor("weight", [D_OUT, D_IN], F32, kind="ExternalInput")
    score = nc.dram_tensor("score", [D_OUT, D_IN], F32, kind="ExternalInput")

    out_sh = nc.dram_tensor("out_shard", [M_SH, O_SH], F32, kind="ExternalOutput")
    assoc_sh = nc.dram_tensor("assoc_shard", [O_SH, D_IN], F32, kind="ExternalOutput")
    maskrow = nc.dram_tensor("mask_row", [1, D_IN], F32, kind="ExternalOutput")
    dbg = nc.dram_tensor("dbg", [1, 8], F32, kind="ExternalOutput")

    with tile.TileContext(nc) as tc:
        with (
            tc.tile_pool(name="small", bufs=1) as small,
            tc.tile_pool(name="dram", bufs=1, space="DRAM") as dram,
        ):
            ident = small.tile([128, 128], F32, tag="ident")
            make_identity(nc, ident[:])
            ident_bf = small.tile([128, 128], BF16, tag="ident_bf")
            nc.vector.tensor_copy(ident_bf[:], ident[:])
            ones_neg = small.tile([128, 1], F32, tag="ones_neg")
            nc.vector.memset(ones_neg[:], -1.0 / D_OUT)
            ones_half = small.tile([128, 1], BF16, tag="ones_half")
            nc.vector.memset(ones_half[:], 1.0)
            ones_row = small.tile([1, 128], BF16, tag="ones_row")
            nc.vector.memset(ones_row[:], 1.0)
            jramp = small.tile([128, BIS_WAYS], F32, tag="jramp")
            for j in range(BIS_WAYS):
                nc.vector.memset(jramp[:, j:j + 1], float(j + 1))

            stats_dr = dram.tile([2, D_IN], F32)
            sg_scr = dram.tile([O_SH, D_IN], BF16)

            x_t = small.tile([128, KT], F32, tag="x_t")
            absw_t = small.tile([128, KT], F32, tag="absw_t")
            mask_t = small.tile([128, KT], F32, tag="mask_t")
            s_b = small.tile([128, 1], F32, tag="s_b")
            mask_row_sb = small.tile([1, D_IN], BF16, tag="mask_row_sb")
            mask_bc = small.tile([128, D_IN], BF16, tag="mask_bc")
            # masked sign^T, bf16, resident: [128(k part), KT, O_SH]
            sgnT = small.tile([128, KT, O_SH], BF16, tag="sgnT")

            # ---------------- phase 1: stats + sign + sign^T -----------
            with (
                tc.tile_pool(name="pstat", bufs=2) as pstat,
                tc.tile_pool(name="pacc", bufs=1) as pacc,
                tc.tile_pool(name="ps_stat", bufs=2, space="PSUM") as ps_stat,
                tc.tile_pool(name="ps_tr", bufs=2, space="PSUM") as ps_tr,
            ):
                acc_s = pacc.tile([128, D_IN], F32, tag="acc_s")
                acc_w = pacc.tile([128, D_IN], BF16, tag="acc_w")
                nc.vector.memset(acc_s[:], 0.0)
                nc.vector.memset(acc_w[:], 0.0)
                HW_ = D_IN // 2
                for ot in range(WTS):
                    rows = slice(ot * 128, (ot + 1) * 128)
                    for h in range(2):
                        cols = slice(h * HW_, (h + 1) * HW_)
                        st = pstat.tile([128, HW_], F32, tag="st")
                        wt = pstat.tile([128, HW_], F32, tag="wt")
                        nc.sync.dma_start(st[:], score[rows, cols])
                        nc.sync.dma_start(wt[:], weight[rows, cols])
                        if ot < OTS:
                            # sign(w) of the shard rows, bf16 (exact +-1/0)
                            sgn_bf = pstat.tile([128, HW_], BF16, tag="sgn_bf")
                            nc.scalar.sign(sgn_bf[:], wt[:])
                            nc.sync.dma_start(sg_scr[rows, cols], sgn_bf[:])
                            # transpose shard sign into sgnT via PE (bf16)
                            for kb in range(4):
                                ptr = ps_tr.tile([128, 512], BF16, tag="ptr")
                                for j in range(4):
                                    kq = kb * 4 + j
                                    nc.tensor.matmul(
                                        ptr[:, j * 128:(j + 1) * 128],
                                        sgn_bf[:, kq * 128:(kq + 1) * 128],
                                        ident_bf[:],
                                        is_transpose=True,
                                        start=(j == 0), stop=(j == 3))
                                kt0 = h * 16 + kb * 4
                                nc.scalar.copy(
                                    sgnT[:, kt0:kt0 + 4,
                                         ot * 128:(ot + 1) * 128],
                                    ptr[:].rearrange("p (j m) -> p j m", j=4))
                        # column stats accumulate
                        nc.vector.tensor_tensor(
                            out=acc_s[:, cols], in0=acc_s[:, cols],
                            in1=st[:], op=ALU.add)
                        aw = pstat.tile([128, HW_], BF16, tag="aw")
                        nc.scalar.activation(aw[:], wt[:], ACTF.Abs)
                        nc.vector.tensor_tensor(
                            out=acc_w[:, cols], in0=acc_w[:, cols],
                            in1=aw[:], op=ALU.add)

                # partition-reduce via ones-matmuls, staged to DRAM:
                # row 0: -colsum(score)/D_OUT ; row 1: colsum(|w|)
                for c in range(8):
                    sl = slice(c * 512, (c + 1) * 512)
                    p1 = ps_stat.tile([1, 512], F32, tag="ps_cm")
                    nc.tensor.matmul(p1[:], ones_neg[:], acc_s[:, sl],
                                     start=True, stop=True)
                    stg1 = pstat.tile([1, 512], F32, tag="stage")
                    nc.scalar.copy(stg1[:], p1[:])
                    nc.sync.dma_start(stats_dr[0:1, sl], stg1[:])
                    p2 = ps_stat.tile([1, 512], F32, tag="ps_aw")
                    nc.tensor.matmul(p2[:], ones_half[:], acc_w[:, sl],
                                     start=True, stop=True)
                    stg2 = pstat.tile([1, 512], F32, tag="stage")
                    nc.scalar.copy(stg2[:], p2[:])
                    nc.sync.dma_start(stats_dr[1:2, sl], stg2[:])

                # x = -col_mean in [128, 32] layout with i = f*128 + p
                nc.sync.dma_start(
                    x_t[:],
                    stats_dr[0:1, :].rearrange("o (f p) -> (o p) f", p=128))
                nc.sync.dma_start(
                    absw_t[:],
                    stats_dr[1:2, :].rearrange("o (f p) -> (o p) f", p=128))
                x_row = pacc.tile([1, D_IN], F32, tag="x_row")
                nc.sync.dma_start(x_row[:], stats_dr[0:1, :])

                # ---- threshold via 16-ary bisection on count(x <= t) ----
                lo = small.tile([128, 1], F32, tag="lo")
                dl = small.tile([128, 1], F32, tag="dl")
                tvec = small.tile([128, BIS_WAYS], F32, tag="tvec")
                cmp = small.tile([128, KT], F32, tag="cmp")
                pc = small.tile([128, BIS_WAYS], F32, tag="pc")
                pct = small.tile([128, BIS_WAYS], F32, tag="pct")
                ind = small.tile([128, BIS_WAYS], F32, tag="ind")
                qv = small.tile([128, 1], F32, tag="qv")
                qd = small.tile([128, 1], F32, tag="qd")
                tt = small.tile([128, 1], F32, tag="tt")
                nc.vector.memset(lo[:], BIS_LO)
                nc.vector.memset(dl[:], BIS_RANGE / BIS_WAYS)
                if "nobis" in ablate:
                    nc.vector.memset(tt[:], 1.0e30)
                else:
                    for rnd in range(BIS_ROUNDS):
                        # t_j = (jramp_j * dl) + lo -- two plain rounded ops so
                        # that lo' = (q * dl) + lo below is bitwise t_q
                        nc.vector.tensor_scalar(tvec[:], jramp[:], dl[:],
                                                None, op0=ALU.mult)
                        nc.vector.tensor_scalar(tvec[:], tvec[:], lo[:],
                                                None, op0=ALU.add)
                        for j in range(BIS_WAYS):
                            nc.vector.tensor_scalar(
                                cmp[:], x_t[:], tvec[:, j:j + 1], 0.0,
                                op0=ALU.is_le, op1=ALU.add,
                                accum_out=pc[:, j:j + 1])
                        nc.gpsimd.partition_all_reduce(
                            pct[:], pc[:], channels=128,
                            reduce_op=bass_isa.ReduceOp.add)
                        # q = #(count_j <= N_KEEP - 0.5)
                        nc.vector.tensor_scalar(
                            ind[:], pct[:], N_KEEP - 0.5, 0.0,
                            op0=ALU.is_le, op1=ALU.add, accum_out=qv[:])
                        # lo += q * dl  (same fl sequence as t_q); dl /= 16
                        nc.vector.tensor_tensor(
                            out=qd[:], in0=qv[:], in1=dl[:], op=ALU.mult)
                        nc.vector.tensor_tensor(
                            out=lo[:], in0=qd[:], in1=lo[:], op=ALU.add)
                        nc.vector.tensor_scalar_mul(dl[:], dl[:],
                                                    1.0 / BIS_WAYS)
                    # T = 16*dl + lo (top of final window)
                    nc.vector.tensor_scalar_mul(qd[:], dl[:], float(BIS_WAYS))
                    nc.vector.tensor_tensor(out=tt[:], in0=qd[:], in1=lo[:],
                                            op=ALU.add)

                # mask (keep) = x <= T, with 1-ulp stall guard
                cnt_p = small.tile([128, 1], F32, tag="cnt_p")
                nc.vector.tensor_scalar(mask_t[:], x_t[:], tt[:], 0.0,
                                        op0=ALU.is_le, op1=ALU.add,
                                        accum_out=cnt_p[:])
                cnt = small.tile([128, 1], F32, tag="cnt")
                nc.gpsimd.partition_all_reduce(
                    cnt[:], cnt_p[:], channels=128,
                    reduce_op=bass_isa.ReduceOp.add)
                t2 = small.tile([128, 1], F32, tag="t2")
                nc.vector.tensor_scalar(t2[:], tt[:], 1.0, GUARD_BUMP,
                                        op0=ALU.mult, op1=ALU.add)
                mask2 = small.tile([128, KT], F32, tag="mask2")
                cnt2_p = small.tile([128, 1], F32, tag="cnt2_p")
                nc.vector.tensor_scalar(mask2[:], x_t[:], t2[:], 0.0,
                                        op0=ALU.is_le, op1=ALU.add,
                                        accum_out=cnt2_p[:])
                cnt2 = small.tile([128, 1], F32, tag="cnt2")
                nc.gpsimd.partition_all_reduce(
                    cnt2[:], cnt2_p[:], channels=128,
                    reduce_op=bass_isa.ReduceOp.add)
                sel = small.tile([128, 1], F32, tag="sel")
                nc.vector.tensor_scalar(sel[:], cnt[:], float(N_KEEP), 0.0,
                                        op0=ALU.is_equal, op1=ALU.add)
                selc = small.tile([128, 1], F32, tag="selc")
                nc.vector.tensor_scalar(selc[:], sel[:], -1.0, 1.0,
                                        op0=ALU.mult, op1=ALU.add)
                # mask = sel*mask + (1-sel)*mask2 ; cnt likewise
                nc.vector.tensor_scalar(mask_t[:], mask_t[:], sel[:], None,
                                        op0=ALU.mult)
                nc.vector.scalar_tensor_tensor(
                    out=mask_t[:], in0=mask2[:], scalar=selc[:], in1=mask_t[:],
                    op0=ALU.mult, op1=ALU.add)
                nc.vector.tensor_scalar(cnt[:], cnt[:], sel[:], None,
                                        op0=ALU.mult)
                nc.vector.scalar_tensor_tensor(
                    out=cnt[:], in0=cnt2[:], scalar=selc[:], in1=cnt[:],
                    op0=ALU.mult, op1=ALU.add)
                # guarded threshold for the row-layout mask
                tg = small.tile([128, 1], F32, tag="tg")
                nc.vector.tensor_scalar(tg[:], tt[:], sel[:], None,
                                        op0=ALU.mult)
                nc.vector.scalar_tensor_tensor(
                    out=tg[:], in0=t2[:], scalar=selc[:], in1=tg[:],
                    op0=ALU.mult, op1=ALU.add)

                # num = sum(mask * absw)
                num_t = small.tile([128, KT], F32, tag="num_t")
                num_p = small.tile([128, 1], F32, tag="num_p")
                nc.vector.scalar_tensor_tensor(
                    out=num_t[:], in0=mask_t[:], scalar=0.0, in1=absw_t[:],
                    op0=ALU.bypass, op1=ALU.mult, accum_out=num_p[:])
                num = small.tile([128, 1], F32, tag="num")
                nc.gpsimd.partition_all_reduce(
                    num[:], num_p[:], channels=128,
                    reduce_op=bass_isa.ReduceOp.add)
                # scaling = num / (cnt * D_OUT): reciprocal + one Newton step
                d = small.tile([128, 1], F32, tag="d")
                nc.vector.tensor_scalar_mul(d[:], cnt[:], float(D_OUT))
                r0 = small.tile([128, 1], F32, tag="r0")
                nc.vector.reciprocal(r0[:], d[:])
                e = small.tile([128, 1], F32, tag="e")
                nc.vector.tensor_tensor(out=e[:], in0=d[:], in1=r0[:],
                                        op=ALU.mult)
                f2 = small.tile([128, 1], F32, tag="f2")
                nc.vector.tensor_scalar(f2[:], e[:], -1.0, 2.0, op0=ALU.mult,
                                        op1=ALU.add)
                rr = small.tile([128, 1], F32, tag="rr")
                nc.vector.tensor_tensor(out=rr[:], in0=r0[:], in1=f2[:],
                                        op=ALU.mult)
                nc.vector.tensor_tensor(out=s_b[:], in0=num[:], in1=rr[:],
                                        op=ALU.mult)

                # mask in row layout (bf16; 0/1 exact)
                nc.vector.tensor_scalar(mask_row_sb[:], x_row[:],
                                        tg[:1, 0:1], None, op0=ALU.is_le)
                # debug scalars
                dbg_sb = small.tile([1, 8], F32, tag="dbg_sb")
                nc.vector.tensor_copy(dbg_sb[:, 0:1], tt[:1, :])
                nc.vector.tensor_copy(dbg_sb[:, 1:2], tg[:1, :])
                nc.vector.tensor_copy(dbg_sb[:, 2:3], cnt[:1, :])
                nc.vector.tensor_copy(dbg_sb[:, 3:4], num[:1, :])
                nc.vector.tensor_copy(dbg_sb[:, 4:5], s_b[:1, :])
                nc.vector.tensor_copy(dbg_sb[:, 5:6], sel[:1, :])
                nc.vector.tensor_copy(dbg_sb[:, 6:7], cnt2[:1, :])
                nc.vector.tensor_copy(dbg_sb[:, 7:8], e[:1, :])
                nc.sync.dma_start(dbg[:], dbg_sb[:])

            # mask broadcast to all partitions (for assoc), bf16
            with tc.tile_pool(name="ps_bc", bufs=2, space="PSUM") as ps_bc:
                for c in range(8):
                    sl = slice(c * 512, (c + 1) * 512)
                    pb = ps_bc.tile([128, 512], F32, tag="pb")
                    nc.tensor.matmul(pb[:], ones_row[:], mask_row_sb[:, sl],
                                     start=True, stop=True)
                    nc.scalar.copy(mask_bc[:, sl], pb[:])

            # ---------------- phase 2: main matmul + assoc -------------
            with (
                tc.tile_pool(name="pmain", bufs=2) as pmain,
                tc.tile_pool(name="pchunk", bufs=3) as pchunk,
                tc.tile_pool(name="pmm", bufs=2, space="PSUM") as pmm,
            ):
                # apply mask to sgnT (per-partition: partition = k)
                for kt in range(KT):
                    nc.vector.tensor_scalar_mul(
                        sgnT[:, kt, :], sgnT[:, kt, :], mask_t[:, kt:kt + 1])

                def assoc_piece(ot, h):
                    rows = slice(ot * 128, (ot + 1) * 128)
                    cols = slice(h * 2048, (h + 1) * 2048)
                    st2 = pmain.tile([128, 2048], F32, tag="st2")
                    nc.sync.dma_start(st2[:], score[rows, cols])
                    sgn_n = pmain.tile([128, 2048], BF16, tag="sgn_n")
                    nc.sync.dma_start(sgn_n[:], sg_scr[rows, cols])
                    pm = pmain.tile([128, 2048], F32, tag="pm")
                    nc.vector.tensor_tensor(out=pm[:], in0=mask_bc[:, cols],
                                            in1=st2[:], op=ALU.subtract)
                    nc.vector.tensor_tensor(out=pm[:], in0=pm[:], in1=st2[:],
                                            op=ALU.add)
                    if ot == 0:
                        nc.sync.dma_start(maskrow[0:1, cols], pm[0:1, :])
                    nc.vector.tensor_tensor(out=pm[:], in0=sgn_n[:], in1=pm[:],
                                            op=ALU.mult)
                    nc.sync.dma_start(assoc_sh[rows, cols], pm[:])

                for rep in range(reps):
                  assoc_jobs = [(ot, h) for ot in range(OTS) for h in range(2)]
                  for mt in range(MT):
                    hiT = pmain.tile([128, D_IN], F16, tag="hiT")
                    loT = pmain.tile([128, D_IN], BF16, tag="loT")
                    mrows = slice(mt * 128, (mt + 1) * 128)
                    if "notr" in ablate:
                        nc.vector.memset(hiT[:], 0.0)
                        nc.vector.memset(loT[:], 0.0)
                    else:
                        for q in range(8):
                            sc = pchunk.tile([128, 512], F32, tag="sc")
                            nc.sync.dma_start(
                                sc[:], sample[mrows, q * 512:(q + 1) * 512])
                            pt = pmm.tile([128, 512], F32, tag="ps_t")
                            for j in range(4):
                                nc.tensor.matmul(
                                    pt[:, j * 128:(j + 1) * 128],
                                    sc[:, j * 128:(j + 1) * 128],
                                    ident[:],
                                    is_transpose=True,
                                    start=(j == 0), stop=(j == 3))
                            qsl = slice(q * 512, (q + 1) * 512)
                            nc.scalar.copy(hiT[:, qsl], pt[:])
                            nc.vector.tensor_tensor(
                                out=loT[:, qsl], in0=pt[:], in1=hiT[:, qsl],
                                op=ALU.subtract)
                    for o2 in range(2):
                        po = pmm.tile([128, 512], F32, tag="ps_o")
                        osl = slice(o2 * 512, (o2 + 1) * 512)
                        if "nomm" in ablate:
                            nc.vector.memset(po[:], 0.0)
                        else:
                            for si, split in enumerate((hiT, loT)):
                                for kt in range(KT):
                                    nc.tensor.matmul(
                                        po[:],
                                        split[:, kt * 128:(kt + 1) * 128],
                                        sgnT[:, kt, osl],
                                        start=(si == 0 and kt == 0),
                                        stop=(si == 1 and kt == KT - 1))
                        ob = pmain.tile([128, 512], F32, tag="ob")
                        nc.scalar.activation(ob[:], po[:], ACTF.Copy,
                                             scale=s_b[:])
                        nc.sync.dma_start(out_sh[mrows, osl], ob[:])
                    # interleave one assoc piece every other m-tile
                    if "noassoc" not in ablate:
                        if mt % 2 == 0 and assoc_jobs:
                            assoc_piece(*assoc_jobs.pop(0))
                if "noassoc" not in ablate:
                    while assoc_jobs:
                        assoc_piece(*assoc_jobs.pop(0))

    nc.compile()
    return nc


def _get_nc():
    global _NC
    if _NC is None:
        _NC = _build()
    return _NC


LAST_RUN_INFO = {}


def _rotated(a, jn):
    """Full array with rows rotated so shard jn sits at rows [0, O_SH)."""
    if jn == 0:
        return a
    return np.ascontiguousarray(np.roll(a, -jn * O_SH, axis=0))


def kernel(sample, weight, score):
    import time
    nc = _get_nc()
    sample2d = np.ascontiguousarray(
        np.asarray(sample, dtype=np.float32).reshape(M, D_IN))
    weight = np.asarray(weight, dtype=np.float32)
    score = np.asarray(score, dtype=np.float32)

    wrot = [_rotated(weight, jn) for jn in range(GN)]
    srot = [_rotated(score, jn) for jn in range(GN)]

    in_maps = []
    for c in range(8):
        im, jn = divmod(c, GN)
        in_maps.append({
            "sample": sample2d[im * M_SH:(im + 1) * M_SH],
            "weight": wrot[jn],
            "score": srot[jn],
        })

    t0 = time.time()
    res = run_bass_kernel_spmd(nc, in_maps, core_ids=list(range(8)))
    LAST_RUN_INFO["spmd_wall_s"] = time.time() - t0
    r = res.results

    out = np.empty((M, D_OUT), np.float32)
    for c in range(8):
        im, jn = divmod(c, GN)
        out[im * M_SH:(im + 1) * M_SH,
            jn * O_SH:(jn + 1) * O_SH] = r[c]["out_shard"]
    assoc = np.concatenate([r[jn]["assoc_shard"] for jn in range(GN)], axis=0)
    mask_row = r[0]["mask_row"][0]
    LAST_RUN_INFO["dbg"] = r[0]["dbg"]
    return (out.reshape(B, S, D_OUT), assoc, mask_row)
